# revision 14
# baseline (speedup 1.0000x reference)
"""Cross-attention (softmax over queries) on 8 Trainium2 NeuronCores.

Reference (per batch b):
    q = y @ Wq.T + bq            [N, H]
    k = x @ Wk.T + bk            [M, H]
    v = x @ Wv.T + bv            [M, D]
    dots = (q @ k.T) * H**-0.5   [N, M]
    attn = softmax(dots, axis=0) (over queries n, per key column m)
    out  = attn @ v              [N, D]

Sharding: data-parallel over batch B=8, one batch per core (SPMD).

Device algorithm (per core, all matmuls fp16 with fp32 PSUM accumulation):
  A. PE-transpose 128x128 f32 blocks of y,x (identity matmul) and cast to fp16
     on the PSUM->SBUF copy (ACT), giving yT[c,n], xT[c,m]; project qT[h,n],
     kT[h,m], v[m,d] (weights arrive pre-transposed/pre-scaled fp16 from
     host; biases added via K=1 matmuls).
  C. per 128-row key chunk: dotsT[m,n] in PSUM, column max (DVE), fused
     exp+rowsum on ACT into attnT fp16, fold 1/sum into v rows.
  D. out[n,d] = sum_m attnT[m,n] * v'[m,d], accumulate over 16 m-chunks.
"""

import numpy as np

import concourse.mybir as mybir
import concourse.tile as tile
from concourse import bacc
from concourse.bass_utils import run_bass_kernel_spmd
from concourse.masks import make_identity

F32 = mybir.dt.float32
F16 = mybir.dt.float16
Exp = mybir.ActivationFunctionType.Exp
AX = mybir.AxisListType.X

B, N, M, C, H, D = 8, 2048, 2048, 1024, 512, 1024
P = 128
NT, MT, CCH, HC = N // P, M // P, C // P, H // P  # 16, 16, 8, 4
SCALE = (C // 2) ** -0.5

_CACHE = {}


def _build_nc():
    nc = bacc.Bacc("TRN2", target_bir_lowering=False, debug=False)

    y_d = nc.dram_tensor("y", [N, C], F32, kind="ExternalInput").ap()
    x_d = nc.dram_tensor("x", [M, C], F32, kind="ExternalInput").ap()
    wqt_d = nc.dram_tensor("wqt", [C, H], F16, kind="ExternalInput").ap()
    wkt_d = nc.dram_tensor("wkt", [C, H], F16, kind="ExternalInput").ap()
    wvt_d = nc.dram_tensor("wvt", [C, D], F16, kind="ExternalInput").ap()
    bq_d = nc.dram_tensor("bq", [H], F32, kind="ExternalInput").ap()
    bk_d = nc.dram_tensor("bk", [H], F32, kind="ExternalInput").ap()
    bv_d = nc.dram_tensor("bv", [D], F16, kind="ExternalInput").ap()
    out_d = nc.dram_tensor("out", [N, D], F32, kind="ExternalOutput").ap()

    y_r = y_d.rearrange("(t p) c -> p t c", p=P)  # [128, 16, 1024]
    x_r = x_d.rearrange("(t p) c -> p t c", p=P)
    out_r = out_d.rearrange("(t p) d -> p t d", p=P)

    with tile.TileContext(nc) as tc:
        with (
            tc.tile_pool(name="persist", bufs=1) as pers,
            tc.tile_pool(name="stats", bufs=1) as stats,
        ):
            qT = pers.tile([P, HC, N], F16, tag="qT")  # [h%128, h//128, n] 2MB
            kT = pers.tile([P, HC, M], F16, tag="kT")  # 2MB
            v = pers.tile([P, MT, D], F16, tag="v")  # [m%128, m//128, d] 4MB
            attnT = pers.tile([P, MT, N], F16, tag="attnT")  # 8MB
            ones = pers.tile([1, 512], F16, tag="ones")
            nc.vector.memset(ones[:], 1.0)
            ident = pers.tile([P, P], F32, tag="ident")
            make_identity(nc, ident[:])

            sums = stats.tile([P, MT], F32, tag="sums")
            rsum = stats.tile([P, MT], F32, tag="rsum")

            # ---------- Phase A: transposes + projections ----------
            with (
                tc.tile_pool(name="stage_t", bufs=1) as sa,
                tc.tile_pool(name="stage_ld", bufs=2) as sld,
                tc.tile_pool(name="wq_pool", bufs=1) as wqp,
                tc.tile_pool(name="wk_pool", bufs=1) as wkp,
                tc.tile_pool(name="ps_a", bufs=4, space="PSUM") as psA,
            ):
                # wq and wv share one 2MB slot (tag "wqv"); wv allocates after
                # the qT projection releases wq.
                wq_sb = wqp.tile([P, CCH, H], F16, tag="wqv")  # [c%128, c//128, h]
                wk_sb = wkp.tile([P, CCH, H], F16, tag="wk")
                bq_sb = wqp.tile([P, HC], F32, tag="bq")  # per-partition bias [h%128, h//128]
                bk_sb = wkp.tile([P, HC], F32, tag="bk")
                bv_sb = wkp.tile([1, D], F16, tag="bv")
                nc.sync.dma_start(wq_sb[:], wqt_d.rearrange("(o p) h -> p o h", p=P))
                nc.sync.dma_start(wk_sb[:], wkt_d.rearrange("(o p) h -> p o h", p=P))
                nc.sync.dma_start(bq_sb[:], bq_d.rearrange("(o p) -> p o", p=P))
                nc.sync.dma_start(bk_sb[:], bk_d.rearrange("(o p) -> p o", p=P))
                nc.sync.dma_start(bv_sb[:], bv_d[None, :])

                def load_transposed(src_r, dst, tag):
                    # src_r: DRAM [128, 16, 1024] f32; dst: SBUF [128, 8, 2048] f16
                    # PE-transposes 128x128 f32 blocks; ACT copy casts to f16.
                    for quarter in range(4):
                        a32 = sld.tile([P, 4, C], F32, tag=tag)
                        nc.sync.dma_start(
                            out=a32[:], in_=src_r[:, quarter * 4 : quarter * 4 + 4, :]
                        )
                        for cc in range(CCH):
                            ptr = psA.tile([P, 512], F32, tag="tr")
                            for t4 in range(4):
                                nc.tensor.transpose(
                                    ptr[:, t4 * P : (t4 + 1) * P],
                                    a32[:, t4, cc * P : (cc + 1) * P],
                                    ident[:],
                                )
                            nc.scalar.copy(
                                dst[:, cc, quarter * 512 : (quarter + 1) * 512], ptr[:]
                            )

                def project(dst, w_sb, b_sb, src_T, n_cols, hcs):
                    # dst[:, hc, j*512:...] = sum_cc w_sb[:,cc,hc*128:...]^T @ src_T[:,cc,j*512:...] + b[hc*128:...]
                    for hc in range(hcs):
                        for j in range(n_cols // 512):
                            pp = psA.tile([P, 512], F32, tag="pp")
                            for cc in range(CCH):
                                nc.tensor.matmul(
                                    pp[:],
                                    w_sb[:, cc, hc * P : (hc + 1) * P],
                                    src_T[:, cc, j * 512 : (j + 1) * 512],
                                    start=(cc == 0),
                                    stop=(cc == CCH - 1),
                                )
                            # ACT copy with per-partition bias add (+ f16 cast)
                            nc.scalar.add(
                                dst[:, hc, j * 512 : (j + 1) * 512],
                                pp[:],
                                b_sb[:, hc : hc + 1],
                            )

                # y -> yT -> qT
                yT = sa.tile([P, CCH, N], F16, tag="actT")  # [c%128, c//128, n] 4MB
                load_transposed(y_r, yT, "act16")
                project(qT, wq_sb, bq_sb, yT, N, HC)

                # x -> xT -> kT, v
                xT = sa.tile([P, CCH, M], F16, tag="actT")
                load_transposed(x_r, xT, "act16")
                project(kT, wk_sb, bk_sb, xT, M, HC)

                wv_sb = wqp.tile([P, CCH, D], F16, tag="wqv")  # 2MB
                nc.sync.dma_start(wv_sb[:], wvt_d.rearrange("(o p) d -> p o d", p=P))
                # v[m, d]: lhsT = xT[:, cc, mc*128:...] (c,m), rhs = wv (c,d)
                for mc in range(MT):
                    for dh in range(2):
                        pv = psA.tile([P, 512], F32, tag="pp")
                        for cc in range(CCH):
                            nc.tensor.matmul(
                                pv[:],
                                xT[:, cc, mc * P : (mc + 1) * P],
                                wv_sb[:, cc, dh * 512 : (dh + 1) * 512],
                                start=(cc == 0),
                                stop=False,
                            )
                        nc.tensor.matmul(
                            pv[:],
                            ones[:, :P],
                            bv_sb[:, dh * 512 : (dh + 1) * 512],
                            start=False,
                            stop=True,
                        )
                        nc.scalar.copy(v[:, mc, dh * 512 : (dh + 1) * 512], pv[:])

            # ---------- Phase C: dots + softmax-over-queries ----------
            with (
                tc.tile_pool(name="ps_c", bufs=1, space="PSUM") as psC,
                tc.tile_pool(name="sc", bufs=4) as sc,
            ):
                for mc in range(MT):
                    pd = psC.tile([P, N], F32, tag=f"dots{mc % 2}")  # 4 banks
                    for j in range(N // 512):
                        for hc in range(HC):
                            nc.tensor.matmul(
                                pd[:, j * 512 : (j + 1) * 512],
                                kT[:, hc, mc * P : (mc + 1) * P],
                                qT[:, hc, j * 512 : (j + 1) * 512],
                                start=(hc == 0),
                                stop=(hc == HC - 1),
                            )
                    pmax = sc.tile([P, 4], F32, tag="pmax")
                    for j in range(N // 512):
                        nc.vector.reduce_max(
                            pmax[:, j : j + 1], pd[:, j * 512 : (j + 1) * 512], axis=AX
                        )
                    negmax = sc.tile([P, 1], F32, tag="negmax")
                    nc.vector.reduce_max(negmax[:], pmax[:], axis=AX, negate=True)
                    nc.scalar.activation(
                        out=attnT[:, mc, :],
                        in_=pd[:],
                        func=Exp,
                        bias=negmax[:],
                        accum_out=sums[:, mc : mc + 1],
                    )
                    nc.vector.reciprocal(rsum[:, mc : mc + 1], sums[:, mc : mc + 1])
                    # fold 1/colsum into v rows for this m-chunk
                    nc.vector.tensor_tensor(
                        v[:, mc, :],
                        v[:, mc, :],
                        rsum[:, mc : mc + 1].to_broadcast((P, D)),
                        mybir.AluOpType.mult,
                    )

            # ---------- Phase D: out = attnT^T @ v' ----------
            with (
                tc.tile_pool(name="ps_d", bufs=4, space="PSUM") as psD,
                tc.tile_pool(name="so", bufs=4) as so,
            ):
                for ntc in range(NT):
                    for dh in range(2):
                        po = psD.tile([P, 512], F32, tag="po")
                        for mc in range(MT):
                            nc.tensor.matmul(
                                po[:],
                                attnT[:, mc, ntc * P : (ntc + 1) * P],
                                v[:, mc, dh * 512 : (dh + 1) * 512],
                                start=(mc == 0),
                                stop=(mc == MT - 1),
                            )
                        ot = so.tile([P, 512], F32, tag="ot")
                        nc.scalar.copy(ot[:], po[:])
                        nc.sync.dma_start(
                            out_r[:, ntc, dh * 512 : (dh + 1) * 512], ot[:]
                        )

    nc.finalize()
    return nc


def _get_nc():
    if "nc" not in _CACHE:
        _CACHE["nc"] = _build_nc()
    return _CACHE["nc"]


def _prep_in_maps(y, x, Wq, bq, Wk, bk, Wv, bv):
    y = np.ascontiguousarray(np.asarray(y, dtype=np.float32))
    x = np.ascontiguousarray(np.asarray(x, dtype=np.float32))
    wqt = np.ascontiguousarray((np.asarray(Wq) * SCALE).T.astype(np.float16))
    wkt = np.ascontiguousarray(np.asarray(Wk).T.astype(np.float16))
    wvt = np.ascontiguousarray(np.asarray(Wv).T.astype(np.float16))
    bq32 = (np.asarray(bq) * SCALE).astype(np.float32)
    bk32 = np.asarray(bk, dtype=np.float32)
    bv16 = np.asarray(bv).astype(np.float16)
    return [
        {
            "y": y[b],
            "x": x[b],
            "wqt": wqt,
            "wkt": wkt,
            "wvt": wvt,
            "bq": bq32,
            "bk": bk32,
            "bv": bv16,
        }
        for b in range(B)
    ]


def run(inputs, trace=False, trace_cores=None):
    nc = _get_nc()
    in_maps = _prep_in_maps(**inputs)
    r = run_bass_kernel_spmd(
        nc, in_maps, list(range(B)), trace=trace, trace_cores=trace_cores
    )
    out = np.stack([r.results[b]["out"] for b in range(B)], axis=0)
    return out, r


def kernel(**inputs) -> np.ndarray:
    out, _ = run(inputs, trace=False)
    return out


# revision 23
# speedup vs baseline: 1.0937x; 1.0937x over previous
"""Cross-attention (softmax over queries) on 8 Trainium2 NeuronCores.

Reference (per batch b):
    q = y @ Wq.T + bq            [N, H]
    k = x @ Wk.T + bk            [M, H]
    v = x @ Wv.T + bv            [M, D]
    dots = (q @ k.T) * H**-0.5   [N, M]
    attn = softmax(dots, axis=0) (over queries n, per key column m)
    out  = attn @ v              [N, D]

Sharding: data-parallel over batch B=8, one batch per core (SPMD).

Device algorithm (per core, all matmuls fp16 with fp32 PSUM accumulation):
  A. PE-transpose 128x128 f32 blocks of y,x (identity matmul) and cast to fp16
     on the PSUM->SBUF copy (ACT), giving yT[c,n], xT[c,m]; project qT[h,n],
     kT[h,m], v[m,d] (weights arrive pre-transposed/pre-scaled fp16 from
     host; biases added via K=1 matmuls).
  C. per 128-row key chunk: dotsT[m,n] in PSUM, column max (DVE), fused
     exp+rowsum on ACT into attnT fp16, fold 1/sum into v rows.
  D. out[n,d] = sum_m attnT[m,n] * v'[m,d], accumulate over 16 m-chunks.
"""

import numpy as np

import concourse.mybir as mybir
import concourse.tile as tile
from concourse import bacc
from concourse.bass_utils import run_bass_kernel_spmd
from concourse.masks import make_identity

F32 = mybir.dt.float32
F16 = mybir.dt.float16
Exp = mybir.ActivationFunctionType.Exp
AX = mybir.AxisListType.X

B, N, M, C, H, D = 8, 2048, 2048, 1024, 512, 1024
P = 128
NT, MT, CCH, HC = N // P, M // P, C // P, H // P  # 16, 16, 8, 4
SCALE = (C // 2) ** -0.5

_CACHE = {}


def _build_nc():
    nc = bacc.Bacc("TRN2", target_bir_lowering=False, debug=False)

    y_d = nc.dram_tensor("y", [N, C], F32, kind="ExternalInput").ap()
    x_d = nc.dram_tensor("x", [M, C], F32, kind="ExternalInput").ap()
    wqt_d = nc.dram_tensor("wqt", [C, H], F16, kind="ExternalInput").ap()
    wkt_d = nc.dram_tensor("wkt", [C, H], F16, kind="ExternalInput").ap()
    wvt_d = nc.dram_tensor("wvt", [C, D], F16, kind="ExternalInput").ap()
    bq_d = nc.dram_tensor("bq", [H], F32, kind="ExternalInput").ap()
    bk_d = nc.dram_tensor("bk", [H], F32, kind="ExternalInput").ap()
    bv_d = nc.dram_tensor("bv", [D], F16, kind="ExternalInput").ap()
    out_d = nc.dram_tensor("out", [N, D], F32, kind="ExternalOutput").ap()

    y_r = y_d.rearrange("(t p) c -> p t c", p=P)  # [128, 16, 1024]
    x_r = x_d.rearrange("(t p) c -> p t c", p=P)
    out_r = out_d.rearrange("(t p) d -> p t d", p=P)

    with tile.TileContext(nc) as tc:
        with (
            tc.tile_pool(name="persist", bufs=1) as pers,
            tc.tile_pool(name="stats", bufs=1) as stats,
        ):
            qT = pers.tile([P, HC, N], F16, tag="qT")  # [h%128, h//128, n] 2MB
            kT = pers.tile([P, HC, M], F16, tag="kT")  # 2MB
            v = pers.tile([P, MT, D], F16, tag="v")  # [m%128, m//128, d] 4MB
            attnT = pers.tile([P, MT, N], F16, tag="attnT")  # 8MB
            ones = pers.tile([1, 512], F16, tag="ones")
            nc.vector.memset(ones[:], 1.0)
            ident = pers.tile([P, P], F16, tag="ident")
            make_identity(nc, ident[:])

            sums = stats.tile([P, MT], F32, tag="sums")
            rsum = stats.tile([P, MT], F32, tag="rsum")

            # ---------- Phase A: transposes + projections ----------
            with (
                tc.tile_pool(name="stage_t", bufs=1) as sa,
                tc.tile_pool(name="stage_ld", bufs=2) as sld,
                tc.tile_pool(name="wq_pool", bufs=1) as wqp,
                tc.tile_pool(name="wk_pool", bufs=1) as wkp,
                tc.tile_pool(name="ps_a", bufs=4, space="PSUM") as psA,
            ):
                # wq, wk, wv cycle through one 2MB slot (tag "wqv"): each
                # allocates once the previous projection releases the slot.
                wq_sb = wqp.tile([P, CCH, H], F16, tag="wqv")  # [c%128, c//128, h]
                bq_sb = wkp.tile([P, HC], F32, tag="bq")  # per-partition [h%128, h//128]
                bk_sb = wkp.tile([P, HC], F32, tag="bk")
                bv_sb = wkp.tile([1, D], F16, tag="bv")
                nc.sync.dma_start(wq_sb[:], wqt_d.rearrange("(o p) h -> p o h", p=P))
                nc.sync.dma_start(bq_sb[:], bq_d.rearrange("(o p) -> p o", p=P))
                nc.sync.dma_start(bk_sb[:], bk_d.rearrange("(o p) -> p o", p=P))
                nc.sync.dma_start(bv_sb[:], bv_d[None, :])

                def load_transposed(src_r, dst, tag):
                    # src_r: DRAM [128, 16, 1024] f32; dst: SBUF [128, 8, 2048] f16
                    # gpsimd DMA converts f32->f16 in flight; PE-transposes
                    # 128x128 f16 blocks; ACT copies PSUM->SBUF.
                    for quarter in range(4):
                        a16 = sld.tile([P, 4, C], F16, tag=tag)
                        nc.gpsimd.dma_start(
                            out=a16[:], in_=src_r[:, quarter * 4 : quarter * 4 + 4, :]
                        )
                        for cc in range(CCH):
                            ptr = psA.tile([P, 512], F16, tag="tr")
                            for t4 in range(4):
                                nc.tensor.transpose(
                                    ptr[:, t4 * P : (t4 + 1) * P],
                                    a16[:, t4, cc * P : (cc + 1) * P],
                                    ident[:],
                                )
                            nc.scalar.copy(
                                dst[:, cc, quarter * 512 : (quarter + 1) * 512], ptr[:]
                            )

                def project(dst, w_sb, b_sb, src_T, n_cols, hcs):
                    # dst[:, hc, j*512:...] = sum_cc w_sb[:,cc,hc*128:...]^T @ src_T[:,cc,j*512:...] + b[hc*128:...]
                    for hc in range(hcs):
                        for j in range(n_cols // 512):
                            pp = psA.tile([P, 512], F32, tag="pp")
                            for cc in range(CCH):
                                nc.tensor.matmul(
                                    pp[:],
                                    w_sb[:, cc, hc * P : (hc + 1) * P],
                                    src_T[:, cc, j * 512 : (j + 1) * 512],
                                    start=(cc == 0),
                                    stop=(cc == CCH - 1),
                                )
                            # ACT copy with per-partition bias add (+ f16 cast)
                            nc.scalar.add(
                                dst[:, hc, j * 512 : (j + 1) * 512],
                                pp[:],
                                b_sb[:, hc : hc + 1],
                            )

                # y -> yT -> qT
                with nc.named_scope("A_y"):
                    yT = sa.tile([P, CCH, N], F16, tag="actT")  # [c%128, c//128, n]
                    load_transposed(y_r, yT, "act16")
                with nc.named_scope("A_qT"):
                    project(qT, wq_sb, bq_sb, yT, N, HC)

                # x -> xT -> kT, v
                with nc.named_scope("A_x"):
                    xT = sa.tile([P, CCH, M], F16, tag="actT")
                    load_transposed(x_r, xT, "act16")
                with nc.named_scope("A_kT"):
                    wk_sb = wqp.tile([P, CCH, H], F16, tag="wqv")
                    nc.sync.dma_start(wk_sb[:], wkt_d.rearrange("(o p) h -> p o h", p=P))
                    project(kT, wk_sb, bk_sb, xT, M, HC)

                wv_sb = wqp.tile([P, CCH, D], F16, tag="wqv")  # 2MB
                nc.sync.dma_start(wv_sb[:], wvt_d.rearrange("(o p) d -> p o d", p=P))
                # v[m, d]: lhsT = xT[:, cc, mc*128:...] (c,m), rhs = wv (c,d)
                with nc.named_scope("A_v"):
                    for mc in range(MT):
                        for dh in range(2):
                            pv = psA.tile([P, 512], F32, tag="pp")
                            for cc in range(CCH):
                                nc.tensor.matmul(
                                    pv[:],
                                    xT[:, cc, mc * P : (mc + 1) * P],
                                    wv_sb[:, cc, dh * 512 : (dh + 1) * 512],
                                    start=(cc == 0),
                                    stop=False,
                                )
                            nc.tensor.matmul(
                                pv[:],
                                ones[:, :P],
                                bv_sb[:, dh * 512 : (dh + 1) * 512],
                                start=False,
                                stop=True,
                            )
                            nc.scalar.copy(v[:, mc, dh * 512 : (dh + 1) * 512], pv[:])

            # ---------- Phase C: dots + softmax-over-queries ----------
            with (
                tc.tile_pool(name="ps_c", bufs=1, space="PSUM") as psC,
                tc.tile_pool(name="sc", bufs=4) as sc,
                nc.named_scope("C_dots"),
            ):
                for mc in range(MT):
                    pd = psC.tile([P, N], F32, tag=f"dots{mc % 2}")  # 4 banks
                    for j in range(N // 512):
                        for hc in range(HC):
                            nc.tensor.matmul(
                                pd[:, j * 512 : (j + 1) * 512],
                                kT[:, hc, mc * P : (mc + 1) * P],
                                qT[:, hc, j * 512 : (j + 1) * 512],
                                start=(hc == 0),
                                stop=(hc == HC - 1),
                            )
                    pmax = sc.tile([P, 4], F32, tag="pmax")
                    for j in range(N // 512):
                        nc.vector.reduce_max(
                            pmax[:, j : j + 1], pd[:, j * 512 : (j + 1) * 512], axis=AX
                        )
                    negmax = sc.tile([P, 1], F32, tag="negmax")
                    nc.vector.reduce_max(negmax[:], pmax[:], axis=AX, negate=True)
                    nc.scalar.activation(
                        out=attnT[:, mc, :],
                        in_=pd[:],
                        func=Exp,
                        bias=negmax[:],
                        accum_out=sums[:, mc : mc + 1],
                    )
                    nc.vector.reciprocal(rsum[:, mc : mc + 1], sums[:, mc : mc + 1])
                    # fold 1/colsum into v rows for this m-chunk
                    nc.vector.tensor_tensor(
                        v[:, mc, :],
                        v[:, mc, :],
                        rsum[:, mc : mc + 1].to_broadcast((P, D)),
                        mybir.AluOpType.mult,
                    )

            # ---------- Phase D: out = attnT^T @ v' ----------
            with (
                tc.tile_pool(name="ps_d", bufs=4, space="PSUM") as psD,
                tc.tile_pool(name="so", bufs=4) as so,
                nc.named_scope("D_out"),
            ):
                for ntc in range(NT):
                    for dh in range(2):
                        po = psD.tile([P, 512], F32, tag="po")
                        for mc in range(MT):
                            nc.tensor.matmul(
                                po[:],
                                attnT[:, mc, ntc * P : (ntc + 1) * P],
                                v[:, mc, dh * 512 : (dh + 1) * 512],
                                start=(mc == 0),
                                stop=(mc == MT - 1),
                            )
                        ot = so.tile([P, 512], F32, tag="ot")
                        nc.scalar.copy(ot[:], po[:])
                        nc.sync.dma_start(
                            out_r[:, ntc, dh * 512 : (dh + 1) * 512], ot[:]
                        )

    nc.finalize()
    return nc


def _get_nc():
    if "nc" not in _CACHE:
        _CACHE["nc"] = _build_nc()
    return _CACHE["nc"]


def _prep_in_maps(y, x, Wq, bq, Wk, bk, Wv, bv):
    y = np.ascontiguousarray(np.asarray(y, dtype=np.float32))
    x = np.ascontiguousarray(np.asarray(x, dtype=np.float32))
    wqt = np.ascontiguousarray((np.asarray(Wq) * SCALE).T.astype(np.float16))
    wkt = np.ascontiguousarray(np.asarray(Wk).T.astype(np.float16))
    wvt = np.ascontiguousarray(np.asarray(Wv).T.astype(np.float16))
    bq32 = (np.asarray(bq) * SCALE).astype(np.float32)
    bk32 = np.asarray(bk, dtype=np.float32)
    bv16 = np.asarray(bv).astype(np.float16)
    return [
        {
            "y": y[b],
            "x": x[b],
            "wqt": wqt,
            "wkt": wkt,
            "wvt": wvt,
            "bq": bq32,
            "bk": bk32,
            "bv": bv16,
        }
        for b in range(B)
    ]


def run(inputs, trace=False, trace_cores=None):
    nc = _get_nc()
    in_maps = _prep_in_maps(**inputs)
    r = run_bass_kernel_spmd(
        nc, in_maps, list(range(B)), trace=trace, trace_cores=trace_cores
    )
    out = np.stack([r.results[b]["out"] for b in range(B)], axis=0)
    return out, r


def kernel(**inputs) -> np.ndarray:
    out, _ = run(inputs, trace=False)
    return out


# revision 26
# speedup vs baseline: 1.1437x; 1.0458x over previous
"""Cross-attention (softmax over queries) on 8 Trainium2 NeuronCores.

Reference (per batch b):
    q = y @ Wq.T + bq            [N, H]
    k = x @ Wk.T + bk            [M, H]
    v = x @ Wv.T + bv            [M, D]
    dots = (q @ k.T) * H**-0.5   [N, M]
    attn = softmax(dots, axis=0) (over queries n, per key column m)
    out  = attn @ v              [N, D]

Sharding: data-parallel over batch B=8, one batch per core (SPMD).

Device algorithm (per core, all matmuls fp16 with fp32 PSUM accumulation):
  A. PE-transpose 128x128 f32 blocks of y,x (identity matmul) and cast to fp16
     on the PSUM->SBUF copy (ACT), giving yT[c,n], xT[c,m]; project qT[h,n],
     kT[h,m], v[m,d] (weights arrive pre-transposed/pre-scaled fp16 from
     host; biases added via K=1 matmuls).
  C. per 128-row key chunk: dotsT[m,n] in PSUM, column max (DVE), fused
     exp+rowsum on ACT into attnT fp16, fold 1/sum into v rows.
  D. out[n,d] = sum_m attnT[m,n] * v'[m,d], accumulate over 16 m-chunks.
"""

import numpy as np

import concourse.mybir as mybir
import concourse.tile as tile
from concourse import bacc
from concourse.bass_utils import run_bass_kernel_spmd
from concourse.masks import make_identity

F32 = mybir.dt.float32
F16 = mybir.dt.float16
Exp = mybir.ActivationFunctionType.Exp
AX = mybir.AxisListType.X

B, N, M, C, H, D = 8, 2048, 2048, 1024, 512, 1024
P = 128
NT, MT, CCH, HC = N // P, M // P, C // P, H // P  # 16, 16, 8, 4
SCALE = (C // 2) ** -0.5

_CACHE = {}


def _build_nc():
    nc = bacc.Bacc("TRN2", target_bir_lowering=False, debug=False)

    y_d = nc.dram_tensor("y", [N, C], F32, kind="ExternalInput").ap()
    x_d = nc.dram_tensor("x", [M, C], F32, kind="ExternalInput").ap()
    wqt_d = nc.dram_tensor("wqt", [C, H], F16, kind="ExternalInput").ap()
    wkt_d = nc.dram_tensor("wkt", [C, H], F16, kind="ExternalInput").ap()
    wvt_d = nc.dram_tensor("wvt", [C, D], F16, kind="ExternalInput").ap()
    bq_d = nc.dram_tensor("bq", [H], F32, kind="ExternalInput").ap()
    bk_d = nc.dram_tensor("bk", [H], F32, kind="ExternalInput").ap()
    bv_d = nc.dram_tensor("bv", [D], F16, kind="ExternalInput").ap()
    out_d = nc.dram_tensor("out", [N, D], F32, kind="ExternalOutput").ap()

    y_r = y_d.rearrange("(t p) c -> p t c", p=P)  # [128, 16, 1024]
    x_r = x_d.rearrange("(t p) c -> p t c", p=P)
    out_r = out_d.rearrange("(t p) d -> p t d", p=P)

    with tile.TileContext(nc) as tc:
        with (
            tc.tile_pool(name="persist", bufs=1) as pers,
            tc.tile_pool(name="stats", bufs=1) as stats,
        ):
            qT = pers.tile([P, HC, N], F16, tag="qT")  # [h%128, h//128, n] 2MB
            kT = pers.tile([P, HC, M], F16, tag="kT")  # 2MB
            v = pers.tile([P, MT, D], F16, tag="v")  # [m%128, m//128, d] 4MB
            attnT = pers.tile([P, MT, N], F16, tag="attnT")  # 8MB
            ones = pers.tile([1, 512], F16, tag="ones")
            nc.vector.memset(ones[:], 1.0)
            ident = pers.tile([P, P], F16, tag="ident")
            make_identity(nc, ident[:])

            sums = stats.tile([P, MT], F32, tag="sums")
            rsum = stats.tile([P, MT], F32, tag="rsum")

            # ---------- Phase A: transposes + projections ----------
            with (
                tc.tile_pool(name="stage_t", bufs=1) as sa,
                tc.tile_pool(name="stage_ld", bufs=2) as sld,
                tc.tile_pool(name="wq_pool", bufs=1) as wqp,
                tc.tile_pool(name="wk_pool", bufs=1) as wkp,
                tc.tile_pool(name="ps_a", bufs=4, space="PSUM") as psA,
            ):
                # wq, wk, wv cycle through one 2MB slot (tag "wqv"): each
                # allocates once the previous projection releases the slot.
                wq_sb = wqp.tile([P, CCH, H], F16, tag="wqv")  # [c%128, c//128, h]
                wk_sb = wkp.tile([P, CCH, H], F16, tag="wk")
                bq_sb = wkp.tile([P, HC], F32, tag="bq")  # per-partition [h%128, h//128]
                bk_sb = wkp.tile([P, HC], F32, tag="bk")
                bv_sb = wkp.tile([1, D], F16, tag="bv")
                nc.sync.dma_start(wq_sb[:], wqt_d.rearrange("(o p) h -> p o h", p=P))
                nc.sync.dma_start(wk_sb[:], wkt_d.rearrange("(o p) h -> p o h", p=P))
                nc.sync.dma_start(bq_sb[:], bq_d.rearrange("(o p) -> p o", p=P))
                nc.sync.dma_start(bk_sb[:], bk_d.rearrange("(o p) -> p o", p=P))
                nc.sync.dma_start(bv_sb[:], bv_d[None, :])

                def load_transposed(src_r, dst, tag):
                    # src_r: DRAM [128, 16, 1024] f32; dst: SBUF [128, 8, 2048] f16
                    # gpsimd DMA converts f32->f16 in flight; PE-transposes
                    # 128x128 f16 blocks; ACT copies PSUM->SBUF.
                    for quarter in range(4):
                        a16 = sld.tile([P, 4, C], F16, tag=tag)
                        nc.gpsimd.dma_start(
                            out=a16[:], in_=src_r[:, quarter * 4 : quarter * 4 + 4, :]
                        )
                        for cc in range(CCH):
                            ptr = psA.tile([P, 512], F16, tag="tr")
                            for t4 in range(4):
                                nc.tensor.transpose(
                                    ptr[:, t4 * P : (t4 + 1) * P],
                                    a16[:, t4, cc * P : (cc + 1) * P],
                                    ident[:],
                                )
                            nc.scalar.copy(
                                dst[:, cc, quarter * 512 : (quarter + 1) * 512], ptr[:]
                            )

                def project(dst, w_sb, b_sb, src_T, n_cols, hcs):
                    # dst[:, hc, j*512:...] = sum_cc w_sb[:,cc,hc*128:...]^T @ src_T[:,cc,j*512:...] + b[hc*128:...]
                    for hc in range(hcs):
                        for j in range(n_cols // 512):
                            pp = psA.tile([P, 512], F32, tag="pp")
                            for cc in range(CCH):
                                nc.tensor.matmul(
                                    pp[:],
                                    w_sb[:, cc, hc * P : (hc + 1) * P],
                                    src_T[:, cc, j * 512 : (j + 1) * 512],
                                    start=(cc == 0),
                                    stop=(cc == CCH - 1),
                                )
                            # ACT copy with per-partition bias add (+ f16 cast)
                            nc.scalar.add(
                                dst[:, hc, j * 512 : (j + 1) * 512],
                                pp[:],
                                b_sb[:, hc : hc + 1],
                            )

                # y -> yT -> qT
                with nc.named_scope("A_y"):
                    yT = sa.tile([P, CCH, N], F16, tag="actT")  # [c%128, c//128, n]
                    load_transposed(y_r, yT, "act16")
                with nc.named_scope("A_qT"):
                    project(qT, wq_sb, bq_sb, yT, N, HC)

                # x -> xT -> kT, v
                with nc.named_scope("A_x"):
                    xT = sa.tile([P, CCH, M], F16, tag="actT")
                    load_transposed(x_r, xT, "act16")
                with nc.named_scope("A_kT"):
                    project(kT, wk_sb, bk_sb, xT, M, HC)

                wv_sb = wqp.tile([P, CCH, D], F16, tag="wqv")  # 2MB
                nc.sync.dma_start(wv_sb[:], wvt_d.rearrange("(o p) d -> p o d", p=P))
                # v[m, d]: lhsT = xT[:, cc, mc*128:...] (c,m), rhs = wv (c,d)
                with nc.named_scope("A_v"):
                    for mc in range(MT):
                        for dh in range(2):
                            pv = psA.tile([P, 512], F32, tag="pp")
                            for cc in range(CCH):
                                nc.tensor.matmul(
                                    pv[:],
                                    xT[:, cc, mc * P : (mc + 1) * P],
                                    wv_sb[:, cc, dh * 512 : (dh + 1) * 512],
                                    start=(cc == 0),
                                    stop=False,
                                )
                            nc.tensor.matmul(
                                pv[:],
                                ones[:, :P],
                                bv_sb[:, dh * 512 : (dh + 1) * 512],
                                start=False,
                                stop=True,
                            )
                            nc.scalar.copy(v[:, mc, dh * 512 : (dh + 1) * 512], pv[:])

            # ---------- Phases C+D: dots/softmax, out in two m-halves ----------
            # PSUM: 3 rotating [128,1024] half-row tiles for dotsT (6 banks)
            # + 2 banks for out accumulation chains.
            with (
                tc.tile_pool(name="ps_c", bufs=1, space="PSUM") as psC,
                tc.tile_pool(name="ps_d", bufs=2, space="PSUM") as psD,
                tc.tile_pool(name="late", bufs=1) as late,
                tc.tile_pool(name="sc", bufs=4) as sc,
                tc.tile_pool(name="so", bufs=4) as so,
            ):
                # fp16 partial of out accumulated over m-chunks 0..7
                outp = late.tile([P, NT, D], F16, tag="outp")

                def dots_chunk(mc):
                    halves = []
                    for h in range(2):
                        pd = psC.tile([P, 1024], F32, tag=f"dots{(2 * mc + h) % 3}")
                        for j2 in range(2):
                            j = h * 2 + j2
                            for hc in range(HC):
                                nc.tensor.matmul(
                                    pd[:, j2 * 512 : (j2 + 1) * 512],
                                    kT[:, hc, mc * P : (mc + 1) * P],
                                    qT[:, hc, j * 512 : (j + 1) * 512],
                                    start=(hc == 0),
                                    stop=(hc == HC - 1),
                                )
                        halves.append(pd)
                    pmax = sc.tile([P, 4], F32, tag="pmax")
                    for h in range(2):
                        for j2 in range(2):
                            nc.vector.reduce_max(
                                pmax[:, 2 * h + j2 : 2 * h + j2 + 1],
                                halves[h][:, j2 * 512 : (j2 + 1) * 512],
                                axis=AX,
                            )
                    negmax = sc.tile([P, 1], F32, tag="negmax")
                    nc.vector.reduce_max(negmax[:], pmax[:], axis=AX, negate=True)
                    ssum = sc.tile([P, 2], F32, tag="ssum")
                    for h in range(2):
                        nc.scalar.activation(
                            out=attnT[:, mc, h * 1024 : (h + 1) * 1024],
                            in_=halves[h][:],
                            func=Exp,
                            bias=negmax[:],
                            accum_out=ssum[:, h : h + 1],
                        )
                    nc.vector.tensor_tensor(
                        sums[:, mc : mc + 1],
                        ssum[:, 0:1],
                        ssum[:, 1:2],
                        mybir.AluOpType.add,
                    )
                    nc.vector.reciprocal(rsum[:, mc : mc + 1], sums[:, mc : mc + 1])
                    # fold 1/colsum into v rows for this m-chunk
                    nc.vector.tensor_tensor(
                        v[:, mc, :],
                        v[:, mc, :],
                        rsum[:, mc : mc + 1].to_broadcast((P, D)),
                        mybir.AluOpType.mult,
                    )

                def out_chain(ntc, dh, mc_lo, mc_hi, final):
                    po = psD.tile([P, 512], F32, tag="po")
                    for mc in range(mc_lo, mc_hi):
                        nc.tensor.matmul(
                            po[:],
                            attnT[:, mc, ntc * P : (ntc + 1) * P],
                            v[:, mc, dh * 512 : (dh + 1) * 512],
                            start=(mc == mc_lo),
                            stop=(mc == mc_hi - 1),
                        )
                    if not final:
                        nc.scalar.copy(outp[:, ntc, dh * 512 : (dh + 1) * 512], po[:])
                    else:
                        ot = so.tile([P, 512], F32, tag="ot")
                        nc.vector.tensor_tensor(
                            ot[:],
                            po[:],
                            outp[:, ntc, dh * 512 : (dh + 1) * 512],
                            mybir.AluOpType.add,
                        )
                        nc.sync.dma_start(
                            out_r[:, ntc, dh * 512 : (dh + 1) * 512], ot[:]
                        )

                d_tiles = [(ntc, dh) for ntc in range(NT) for dh in range(2)]

                with nc.named_scope("C_head"):
                    for mc in range(8):
                        dots_chunk(mc)
                with nc.named_scope("CD_mid"):
                    for mc in range(8, MT):
                        dots_chunk(mc)
                        # interleave first-half out chains (m-chunks 0..7)
                        for ntc, dh in d_tiles[(mc - 8) * 4 : (mc - 7) * 4]:
                            out_chain(ntc, dh, 0, 8, final=False)
                with nc.named_scope("D_tail"):
                    for ntc, dh in d_tiles:
                        out_chain(ntc, dh, 8, MT, final=True)

    nc.finalize()
    return nc


def _get_nc():
    if "nc" not in _CACHE:
        _CACHE["nc"] = _build_nc()
    return _CACHE["nc"]


def _prep_in_maps(y, x, Wq, bq, Wk, bk, Wv, bv):
    y = np.ascontiguousarray(np.asarray(y, dtype=np.float32))
    x = np.ascontiguousarray(np.asarray(x, dtype=np.float32))
    wqt = np.ascontiguousarray((np.asarray(Wq) * SCALE).T.astype(np.float16))
    wkt = np.ascontiguousarray(np.asarray(Wk).T.astype(np.float16))
    wvt = np.ascontiguousarray(np.asarray(Wv).T.astype(np.float16))
    bq32 = (np.asarray(bq) * SCALE).astype(np.float32)
    bk32 = np.asarray(bk, dtype=np.float32)
    bv16 = np.asarray(bv).astype(np.float16)
    return [
        {
            "y": y[b],
            "x": x[b],
            "wqt": wqt,
            "wkt": wkt,
            "wvt": wvt,
            "bq": bq32,
            "bk": bk32,
            "bv": bv16,
        }
        for b in range(B)
    ]


def run(inputs, trace=False, trace_cores=None):
    nc = _get_nc()
    in_maps = _prep_in_maps(**inputs)
    r = run_bass_kernel_spmd(
        nc, in_maps, list(range(B)), trace=trace, trace_cores=trace_cores
    )
    out = np.stack([r.results[b]["out"] for b in range(B)], axis=0)
    return out, r


def kernel(**inputs) -> np.ndarray:
    out, _ = run(inputs, trace=False)
    return out


# revision 30
# speedup vs baseline: 1.1868x; 1.0377x over previous
"""Cross-attention (softmax over queries) on 8 Trainium2 NeuronCores.

Reference (per batch b):
    q = y @ Wq.T + bq            [N, H]
    k = x @ Wk.T + bk            [M, H]
    v = x @ Wv.T + bv            [M, D]
    dots = (q @ k.T) * H**-0.5   [N, M]
    attn = softmax(dots, axis=0) (over queries n, per key column m)
    out  = attn @ v              [N, D]

Sharding: data-parallel over batch B=8, one batch per core (SPMD).

Device algorithm (per core, all matmuls fp16 with fp32 PSUM accumulation):
  A. gpsimd DMA casts y,x to fp16 in flight; PE-transpose 128x128 blocks
     (identity matmul) into yT[c,n], xT[c,m]; project qT[h,n], kT[h,m]
     (weights arrive pre-transposed/pre-scaled fp16 from host; q/k biases
     added by the ACT psum->sbuf copy, per-partition).
  C. per 128-row key chunk mc: V-projection chunk (PE filler work, bias via
     K=1 matmul), dotsT[m,n] into two [128,1024] PSUM halves, column max
     (DVE), fused exp+rowsum on ACT into attnT fp16, fold 1/colsum into v.
  D. out[n,d] = sum_m attnT[m,n] * v'[m,d]; dense 16-matmul PSUM chains.
"""

import numpy as np

import concourse.mybir as mybir
import concourse.tile as tile
from concourse import bacc
from concourse.bass_utils import run_bass_kernel_spmd
from concourse.masks import make_identity

F32 = mybir.dt.float32
F16 = mybir.dt.float16
Exp = mybir.ActivationFunctionType.Exp
AX = mybir.AxisListType.X

B, N, M, C, H, D = 8, 2048, 2048, 1024, 512, 1024
P = 128
NT, MT, CCH, HC = N // P, M // P, C // P, H // P  # 16, 16, 8, 4
SCALE = (C // 2) ** -0.5

_CACHE = {}


def _build_nc():
    nc = bacc.Bacc("TRN2", target_bir_lowering=False, debug=False)

    y_d = nc.dram_tensor("y", [N, C], F32, kind="ExternalInput").ap()
    x_d = nc.dram_tensor("x", [M, C], F32, kind="ExternalInput").ap()
    wqt_d = nc.dram_tensor("wqt", [C, H], F16, kind="ExternalInput").ap()
    wkt_d = nc.dram_tensor("wkt", [C, H], F16, kind="ExternalInput").ap()
    wvt_d = nc.dram_tensor("wvt", [C, D], F16, kind="ExternalInput").ap()
    bq_d = nc.dram_tensor("bq", [H], F32, kind="ExternalInput").ap()
    bk_d = nc.dram_tensor("bk", [H], F32, kind="ExternalInput").ap()
    bv_d = nc.dram_tensor("bv", [D], F16, kind="ExternalInput").ap()
    out_d = nc.dram_tensor("out", [N, D], F32, kind="ExternalOutput").ap()

    y_r = y_d.rearrange("(t p) c -> p t c", p=P)  # [128, 16, 1024]
    x_r = x_d.rearrange("(t p) c -> p t c", p=P)
    out_r = out_d.rearrange("(t p) d -> p t d", p=P)

    with tile.TileContext(nc) as tc:
        with (
            tc.tile_pool(name="persist", bufs=1) as pers,
            tc.tile_pool(name="stats", bufs=1) as stats,
            tc.tile_pool(name="xT_pool", bufs=1) as xTp,
            tc.tile_pool(name="ps_pp", bufs=4, space="PSUM") as psPP,
        ):
            qT = pers.tile([P, HC, N], F16, tag="qT")  # [h%128, h//128, n] 2MB
            kT = pers.tile([P, HC, M], F16, tag="kT")  # 2MB
            v = pers.tile([P, MT, D], F16, tag="v")  # [m%128, m//128, d] 4MB
            ones = pers.tile([1, 512], F16, tag="ones")
            nc.vector.memset(ones[:], 1.0)
            ident = pers.tile([P, P], F16, tag="ident")
            make_identity(nc, ident[:])

            sums = stats.tile([P, MT], F32, tag="sums")
            rsum = stats.tile([P, MT], F32, tag="rsum")
            bq_sb = stats.tile([P, HC], F32, tag="bq")  # [h%128, h//128]
            bk_sb = stats.tile([P, HC], F32, tag="bk")
            bv_sb = stats.tile([1, D], F16, tag="bv")
            nc.sync.dma_start(bq_sb[:], bq_d.rearrange("(o p) -> p o", p=P))
            nc.sync.dma_start(bk_sb[:], bk_d.rearrange("(o p) -> p o", p=P))
            nc.sync.dma_start(bv_sb[:], bv_d[None, :])

            xT = xTp.tile([P, CCH, M], F16, tag="xT")  # alive through phase C

            # ---------- Phase A: transposes + q/k projections ----------
            with (
                tc.tile_pool(name="stage_ld", bufs=4) as sld,
                tc.tile_pool(name="yT_pool", bufs=1) as yTp,
                tc.tile_pool(name="w_pool", bufs=1) as wp,
                tc.tile_pool(name="ps_tr", bufs=4, space="PSUM") as psTR,
            ):
                wq_sb = wp.tile([P, CCH, H], F16, tag="wq")  # [c%128, c//128, h]
                wk_sb = wp.tile([P, CCH, H], F16, tag="wk")
                nc.sync.dma_start(wq_sb[:], wqt_d.rearrange("(o p) h -> p o h", p=P))
                nc.sync.dma_start(wk_sb[:], wkt_d.rearrange("(o p) h -> p o h", p=P))

                def load_transposed(src_r, dst, scope):
                    # src_r: DRAM [128, 16, 1024] f32; dst: SBUF [128, 8, 2048] f16
                    # gpsimd DMA converts f32->f16 in flight; PE transposes
                    # 128x128 blocks; ACT copies PSUM->SBUF.
                    with nc.named_scope(scope):
                        for quarter in range(4):
                            a16 = sld.tile([P, 4, C], F16, tag="a16")
                            nc.gpsimd.dma_start(
                                out=a16[:],
                                in_=src_r[:, quarter * 4 : quarter * 4 + 4, :],
                            )
                            for cc in range(CCH):
                                ptr = psTR.tile([P, 512], F16, tag="tr")
                                for t4 in range(4):
                                    nc.tensor.transpose(
                                        ptr[:, t4 * P : (t4 + 1) * P],
                                        a16[:, t4, cc * P : (cc + 1) * P],
                                        ident[:],
                                    )
                                nc.scalar.copy(
                                    dst[:, cc, quarter * 512 : (quarter + 1) * 512],
                                    ptr[:],
                                )

                def project(dst, w_sb, b_sb, src_T, scope):
                    with nc.named_scope(scope):
                        for hc in range(HC):
                            for j in range(N // 512):
                                pp = psPP.tile([P, 512], F32, tag="pp")
                                for cc in range(CCH):
                                    nc.tensor.matmul(
                                        pp[:],
                                        w_sb[:, cc, hc * P : (hc + 1) * P],
                                        src_T[:, cc, j * 512 : (j + 1) * 512],
                                        start=(cc == 0),
                                        stop=(cc == CCH - 1),
                                    )
                                # ACT copy: psum -> f16, + per-partition bias
                                nc.scalar.add(
                                    dst[:, hc, j * 512 : (j + 1) * 512],
                                    pp[:],
                                    b_sb[:, hc : hc + 1],
                                )

                yT = yTp.tile([P, CCH, N], F16, tag="yT")
                load_transposed(y_r, yT, "A_y")
                project(qT, wq_sb, bq_sb, yT, "A_qT")
                load_transposed(x_r, xT, "A_x")
                project(kT, wk_sb, bk_sb, xT, "A_kT")

            # ---------- Phase C: V-proj chunks interleaved with dots/softmax ----------
            with (
                tc.tile_pool(name="late", bufs=1) as late,
                tc.tile_pool(name="sc", bufs=4) as sc,
                tc.tile_pool(name="ps_c", bufs=1, space="PSUM") as psC,
            ):
                attnT = late.tile([P, MT, N], F16, tag="attnT")  # 8MB
                wv_sb = late.tile([P, CCH, D], F16, tag="wv")  # 2MB
                nc.sync.dma_start(wv_sb[:], wvt_d.rearrange("(o p) d -> p o d", p=P))

                def v_chunk(mc):
                    # v[m, d] for m-chunk mc: lhsT = xT (c,m), rhs = wv (c,d)
                    for dh in range(2):
                        pv = psPP.tile([P, 512], F32, tag="pp")
                        for cc in range(CCH):
                            nc.tensor.matmul(
                                pv[:],
                                xT[:, cc, mc * P : (mc + 1) * P],
                                wv_sb[:, cc, dh * 512 : (dh + 1) * 512],
                                start=(cc == 0),
                                stop=False,
                            )
                        nc.tensor.matmul(
                            pv[:],
                            ones[:, :P],
                            bv_sb[:, dh * 512 : (dh + 1) * 512],
                            start=False,
                            stop=True,
                        )
                        nc.scalar.copy(v[:, mc, dh * 512 : (dh + 1) * 512], pv[:])

                def dots_chunk(mc):
                    halves = []
                    for h in range(2):
                        pd = psC.tile([P, 1024], F32, tag=f"dots{h}")
                        for j2 in range(2):
                            j = h * 2 + j2
                            for hc in range(HC):
                                nc.tensor.matmul(
                                    pd[:, j2 * 512 : (j2 + 1) * 512],
                                    kT[:, hc, mc * P : (mc + 1) * P],
                                    qT[:, hc, j * 512 : (j + 1) * 512],
                                    start=(hc == 0),
                                    stop=(hc == HC - 1),
                                )
                        halves.append(pd)
                    pmax = sc.tile([P, 4], F32, tag="pmax")
                    for h in range(2):
                        for j2 in range(2):
                            nc.vector.reduce_max(
                                pmax[:, 2 * h + j2 : 2 * h + j2 + 1],
                                halves[h][:, j2 * 512 : (j2 + 1) * 512],
                                axis=AX,
                            )
                    negmax = sc.tile([P, 1], F32, tag="negmax")
                    nc.vector.reduce_max(negmax[:], pmax[:], axis=AX, negate=True)
                    ssum = sc.tile([P, 2], F32, tag="ssum")
                    for h in range(2):
                        nc.scalar.activation(
                            out=attnT[:, mc, h * 1024 : (h + 1) * 1024],
                            in_=halves[h][:],
                            func=Exp,
                            bias=negmax[:],
                            accum_out=ssum[:, h : h + 1],
                        )
                    nc.vector.tensor_tensor(
                        sums[:, mc : mc + 1],
                        ssum[:, 0:1],
                        ssum[:, 1:2],
                        mybir.AluOpType.add,
                    )
                    nc.vector.reciprocal(rsum[:, mc : mc + 1], sums[:, mc : mc + 1])
                    # fold 1/colsum into v rows for this m-chunk
                    nc.vector.tensor_tensor(
                        v[:, mc, :],
                        v[:, mc, :],
                        rsum[:, mc : mc + 1].to_broadcast((P, D)),
                        mybir.AluOpType.mult,
                    )

                with nc.named_scope("C_loop"):
                    for mc in range(MT):
                        v_chunk(mc)
                        dots_chunk(mc)

                # ---------- Phase D: out = attnT^T @ v' ----------
                with (
                    tc.tile_pool(name="ps_d", bufs=4, space="PSUM") as psD,
                    tc.tile_pool(name="so", bufs=4) as so,
                    nc.named_scope("D_out"),
                ):
                    for ntc in range(NT):
                        for dh in range(2):
                            po = psD.tile([P, 512], F32, tag="po")
                            for mc in range(MT):
                                nc.tensor.matmul(
                                    po[:],
                                    attnT[:, mc, ntc * P : (ntc + 1) * P],
                                    v[:, mc, dh * 512 : (dh + 1) * 512],
                                    start=(mc == 0),
                                    stop=(mc == MT - 1),
                                )
                            ot = so.tile([P, 512], F32, tag="ot")
                            nc.scalar.copy(ot[:], po[:])
                            nc.sync.dma_start(
                                out_r[:, ntc, dh * 512 : (dh + 1) * 512], ot[:]
                            )

    nc.finalize()
    return nc


def _get_nc():
    if "nc" not in _CACHE:
        _CACHE["nc"] = _build_nc()
    return _CACHE["nc"]


def _prep_in_maps(y, x, Wq, bq, Wk, bk, Wv, bv):
    y = np.ascontiguousarray(np.asarray(y, dtype=np.float32))
    x = np.ascontiguousarray(np.asarray(x, dtype=np.float32))
    wqt = np.ascontiguousarray((np.asarray(Wq) * SCALE).T.astype(np.float16))
    wkt = np.ascontiguousarray(np.asarray(Wk).T.astype(np.float16))
    wvt = np.ascontiguousarray(np.asarray(Wv).T.astype(np.float16))
    bq32 = (np.asarray(bq) * SCALE).astype(np.float32)
    bk32 = np.asarray(bk, dtype=np.float32)
    bv16 = np.asarray(bv).astype(np.float16)
    return [
        {
            "y": y[b],
            "x": x[b],
            "wqt": wqt,
            "wkt": wkt,
            "wvt": wvt,
            "bq": bq32,
            "bk": bk32,
            "bv": bv16,
        }
        for b in range(B)
    ]


def run(inputs, trace=False, trace_cores=None):
    nc = _get_nc()
    in_maps = _prep_in_maps(**inputs)
    r = run_bass_kernel_spmd(
        nc, in_maps, list(range(B)), trace=trace, trace_cores=trace_cores
    )
    out = np.stack([r.results[b]["out"] for b in range(B)], axis=0)
    return out, r


def kernel(**inputs) -> np.ndarray:
    out, _ = run(inputs, trace=False)
    return out
